# revision 58
# baseline (speedup 1.0000x reference)
"""MinGRU (2-layer) Trainium2 Bass kernel.

Full shapes: x (8, 4096, 512) f32; w0,w1 (1024, 512); b0,b1 (1024,).
Per layer (reference semantics):
    gh = x @ w.T + b ; gate, hidden = split(gh, 2)
    z = sigmoid(gate); htil = g(hidden) = max(hidden+0.5, sigmoid(hidden))
    h_t = (1-z_t) h_{t-1} + z_t htil_t   (h_0 = 0.5)
Returns (h2, stack([h1[:, -1:], h2[:, -1:]])).

Sharding: data-parallel over batch B=8 across the 8 cores (one batch element
per core, no communication).  On-chip everything is laid out (channel, seq):
matmuls run in float32r (TF32-class accuracy at full PE rate), the gating
sigmoids on the scalar engine read straight out of PSUM with the bias folded
into the activation, and the recurrence is a single hardware
TensorTensorScan per (channel-group, chunk): state = a*state - negb with
negb = (a-1)*g.  x is transposed on-chip via PE matmul-transposes; weights
are pre-transposed (and biases pre-packed) on the host since they are tiny.
DMA instruction count is kept low (one chunk-sized DMA each way per chunk)
because each dma_start pays a fixed HWDGE issue cost.
"""

import os
import numpy as np

import concourse.bacc as bacc
import concourse.tile as tile
import concourse.mybir as mybir
from concourse.bass_utils import run_bass_kernel_spmd
from concourse.masks import make_identity

F32 = mybir.dt.float32
F32R = mybir.dt.float32r
AF = mybir.ActivationFunctionType
OP = mybir.AluOpType

P = 128          # partitions
S = 4096         # sequence length
D = 512          # input dim (layer-0 contraction)
H = 512          # hidden dim
CH = 512         # seq chunk (matmul N, one PSUM bank)
NCH = S // CH    # 8 chunks
KG = D // P      # 4 contraction groups
MT = (2 * H) // P  # 8 output m-tiles (4 gate + 4 hidden)
HG = H // P      # 4 hidden-channel groups
B = 8            # batch (= cores)

# hidden groups whose negb runs on GpSimd (rest on DVE) — load balance
NEGB_POOL_GROUPS = {0, 2}


def build(reps: int = 1):
    nc = bacc.Bacc("TRN2", target_bir_lowering=False, debug=False)
    x_d = nc.dram_tensor("x", [S, D], F32, kind="ExternalInput")
    # host-pretransposed weights: wT[l][g, p, o] = w[l][o, g*128+p]
    wT_d = [
        nc.dram_tensor("wT0", [MT, P, KG, P], F32, kind="ExternalInput"),
        nc.dram_tensor("wT1", [MT, P, KG, P], F32, kind="ExternalInput"),
    ]
    # host-packed biases (128, 24): col = l*12 + kind*4 + i
    # kind: 0 = -b_gate, 1 = b_hidden, 2 = b_hidden+0.5
    bias_d = nc.dram_tensor("biaspack", [P, 2 * 3 * HG], F32, kind="ExternalInput")
    out_d = nc.dram_tensor("out", [S, H], F32, kind="ExternalOutput")
    hN0_d = nc.dram_tensor("hN0", [HG, P], F32, kind="ExternalOutput")

    with tile.TileContext(nc) as tc:
        with (
            tc.tile_pool(name="consts", bufs=1) as consts,
            tc.tile_pool(name="wp", bufs=2 * MT) as wp,
            tc.tile_pool(name="stage", bufs=2) as stage,
            tc.tile_pool(name="xt", bufs=8) as xtp,
            tc.tile_pool(name="h1", bufs=KG * NCH) as h1p,
            tc.tile_pool(name="gat", bufs=6) as gat,
            tc.tile_pool(name="tmp", bufs=3) as tmp,
            tc.tile_pool(name="h2", bufs=8) as h2p,
            tc.tile_pool(name="outT", bufs=2) as outTp,
            tc.tile_pool(name="ps_mm", bufs=8, space="PSUM") as ps_mm,
        ):
            ident = consts.tile([P, P], F32, tag="ident")
            make_identity(nc, ident[:])
            ident_r = consts.tile([P, P], F32R, tag="ident_r")
            nc.scalar.copy(ident_r[:], ident[:])

            btile = consts.tile([P, 2 * 3 * HG], F32, tag="btile")
            nc.sync.dma_start(btile[:], bias_d.ap())

            def bias_col(l, kind, i):
                c = (l * 3 + kind) * HG + i
                return btile[:, c : c + 1]

            wT = [[None] * MT for _ in range(2)]

            def load_w(l):
                # one DMA per m-tile: (128, KG, 128), all k-groups together
                for m in range(MT):
                    t = wp.tile([P, KG, P], F32R, tag="wT", name=f"wT{l}_{m}")
                    nc.sync.dma_start(t[:], wT_d[l].ap()[m].bitcast(F32R))
                    wT[l][m] = t

            def stage_x(c):
                # one DMA: (128, 4, 512) <- x rows [c*512, (c+1)*512)
                t = stage.tile([P, 4, D], F32R, tag="stage", name=f"xst{c}")
                nc.sync.dma_start(
                    t[:],
                    x_d.ap()[c * CH : (c + 1) * CH, :]
                    .rearrange("(j p) d -> p j d", p=P)
                    .bitcast(F32R),
                )
                return t

            def tr_copy(xst, split=False):
                # split=True: xst is a list of 4 per-subblock tiles (chunk 0
                # head-start: each transpose waits on 256KB, not 1MB)
                rhs = []
                for g in range(KG):
                    ptr = ps_mm.tile([P, CH], F32R, tag="gh")
                    for j in range(4):
                        src = (xst[j][:, g * P : (g + 1) * P] if split
                               else xst[:, j, g * P : (g + 1) * P])
                        nc.tensor.transpose(
                            ptr[:, j * P : (j + 1) * P], src, ident_r[:],
                        )
                    xt = xtp.tile([P, CH], F32R, tag="xt")
                    nc.scalar.copy(xt[:], ptr[:])
                    rhs.append(xt)
                return rhs

            def emit_out(c, h2_tiles, last=False):
                ot = outTp.tile([P, 4, H], F32, tag="outT", name=f"ot{c}")
                for j in range(4):
                    ptr = ps_mm.tile([P, CH], F32R, tag="gh", name=f"optr{c}_{j}")
                    for i in range(HG):
                        nc.tensor.transpose(
                            ptr[:, i * P : (i + 1) * P],
                            h2_tiles[i][c][:, j * P : (j + 1) * P],
                            ident_r[:],
                        )
                    if last and j % 2 == 1:
                        nc.vector.tensor_copy(ot[:, j, :], ptr[:])
                    else:
                        nc.scalar.copy(ot[:, j, :], ptr[:])
                    nc.sync.dma_start(
                        out_d.ap()[(c * 4 + j) * P : (c * 4 + j + 1) * P, :],
                        ot[:, j, :],
                    )

            first = True
            LAG = 2  # layer-1 trails layer-0 by this many chunks
            for _rep in range(reps):
                h1_tiles = [[None] * NCH for _ in range(HG)]
                h2_tiles = [[None] * NCH for _ in range(HG)]
                out_pending = []
                xst_q = {}
                rhs_q = {}
                steps = []
                for k in range(NCH + LAG):
                    if k < NCH:
                        steps.append((0, k))
                    if k - LAG >= 0:
                        steps.append((1, k - LAG))
                for l, c in steps:
                        if first:
                            xst0 = []
                            for j in range(4):
                                t0 = stage.tile([P, D], F32R, tag="st0", name=f"xst0_{j}")
                                nc.sync.dma_start(
                                    t0[:],
                                    x_d.ap()[j * P : (j + 1) * P, :].bitcast(F32R),
                                )
                                xst0.append(t0)
                            load_w(0)
                            xst_q[1] = stage_x(1)
                            rhs_q[0] = tr_copy(xst0, split=True)
                            first = False
                        if l == 0 and c == 1:
                            load_w(1)

                        # ---- rhs tiles (128 k, CH) float32r
                        if l == 0:
                            # prefetch DMA two chunks ahead
                            if c + 2 < NCH and (c + 2) not in xst_q:
                                xst_q[c + 2] = stage_x(c + 2)
                            rhs = rhs_q.pop(c)
                        else:
                            rhs = [h1_tiles[g][c] for g in range(KG)]

                        # ---- matmuls: gh m-tiles (128 o, CH s) in PSUM
                        gh = []
                        for m in range(MT):
                            pt = ps_mm.tile([P, CH], F32, tag="gh")
                            for g in range(KG):
                                nc.tensor.matmul(
                                    pt[:],
                                    wT[l][m][:, g, :],
                                    rhs[g][:],
                                    start=(g == 0),
                                    stop=(g == KG - 1),
                                )
                            gh.append(pt)

                        # transpose the next x chunk right after this step's
                        # matmuls (PE spacer while ACT drains the gh tiles)
                        if l == 0 and c + 1 < NCH and (c + 1) not in rhs_q:
                            rhs_q[c + 1] = tr_copy(xst_q.pop(c + 1))

                        # ---- gating + scan per hidden group
                        for i in range(HG):
                            a = gat.tile([P, CH], F32, tag="a")
                            nc.scalar.activation(
                                a[:], gh[i][:], AF.Sigmoid,
                                bias=bias_col(l, 0, i), scale=-1.0,
                            )
                            sh = tmp.tile([P, CH], F32, tag="sh")
                            nc.scalar.activation(
                                sh[:], gh[HG + i][:], AF.Sigmoid,
                                bias=bias_col(l, 1, i), scale=1.0,
                            )
                            gt = tmp.tile([P, CH], F32, tag="g")
                            nc.vector.scalar_tensor_tensor(
                                gt[:], gh[HG + i][:], bias_col(l, 2, i), sh[:],
                                OP.add, OP.max,
                            )
                            negb = gat.tile([P, CH], F32, tag="negb")
                            if i in NEGB_POOL_GROUPS and not (l == 1 and c == NCH - 1):
                                # Pool supports only plain tensor_tensor:
                                # negb = a*g - g = (a-1)*g
                                ag = tmp.tile([P, CH], F32, tag="ag")
                                nc.gpsimd.tensor_tensor(ag[:], a[:], gt[:], OP.mult)
                                nc.gpsimd.tensor_tensor(negb[:], ag[:], gt[:], OP.subtract)
                            else:
                                nc.vector.scalar_tensor_tensor(
                                    negb[:], a[:], 1.0, gt[:], OP.subtract, OP.mult
                                )
                            if l == 0:
                                h = h1p.tile([P, CH], F32R, tag="h1")
                                h1_tiles[i][c] = h
                                init = 0.5 if c == 0 else h1_tiles[i][c - 1][:, CH - 1 : CH].bitcast(F32)
                            else:
                                h = h2p.tile([P, CH], F32R, tag="h2")
                                h2_tiles[i][c] = h
                                init = 0.5 if c == 0 else h2_tiles[i][c - 1][:, CH - 1 : CH].bitcast(F32)
                            nc.vector.tensor_tensor_scan(
                                h[:], a[:], negb[:], init, OP.mult, OP.subtract
                            )

                        # ---- layer-1 output: transpose back to (s, h), store
                        # chunk c-1 (delayed so PE never waits on the scans)
                        if l == 1:
                            out_pending.append(c)
                            if len(out_pending) > 2:
                                emit_out(out_pending.pop(0), h2_tiles)

                while out_pending:
                    c_out = out_pending.pop(0)
                    emit_out(c_out, h2_tiles, last=not out_pending)

                # ---- next-hidden for layer 0 (layer 1's comes from out[:, -1, :])
                hg = consts.tile([P, HG], F32, tag="hgather", name=f"hg{_rep}")
                for i in range(HG):
                    nc.scalar.copy(
                        hg[:, i : i + 1],
                        h1_tiles[i][NCH - 1][:, CH - 1 : CH].bitcast(F32),
                    )
                pgt = ps_mm.tile([HG, P], F32, tag="gh", name=f"pg{_rep}")
                nc.tensor.transpose(pgt[:], hg[:], ident[:])
                hgt = consts.tile([HG, P], F32, tag="hgathT", name=f"hgt{_rep}")
                nc.scalar.copy(hgt[:], pgt[:])
                nc.sync.dma_start(hN0_d.ap(), hgt[:])
    nc.compile()
    return nc


_cache = {}


def _get(reps=1):
    if reps not in _cache:
        _cache[reps] = build(reps)
    return _cache[reps]


def _host_prep(w0, b0, w1, b1):
    """Pre-transpose weights into (KG, 128, 2H) and pack biases (128, 24)."""
    wts = []
    for w in (w0, w1):
        # w.T is (D, 2H): [g*128+p, m*128+o'] -> (MT, P, KG, P) m-major tiles
        wt = np.ascontiguousarray(
            w.T.reshape(KG, P, MT, P).transpose(2, 1, 0, 3)
        )
        wts.append(wt.astype(np.float32))
    bias = np.empty((2, 3, P, HG), np.float32)
    for l, b in enumerate((b0, b1)):
        bg = b[:H].reshape(HG, P).T
        bh = b[H:].reshape(HG, P).T
        bias[l, 0] = -bg
        bias[l, 1] = bh
        bias[l, 2] = bh + 0.5
    biaspack = np.ascontiguousarray(bias.transpose(2, 0, 1, 3).reshape(P, 2 * 3 * HG))
    return wts[0], wts[1], biaspack


def kernel(x, w0, b0, w1, b1):
    x = np.asarray(x, dtype=np.float32)
    w0 = np.asarray(w0, dtype=np.float32)
    b0 = np.asarray(b0, dtype=np.float32)
    w1 = np.asarray(w1, dtype=np.float32)
    b1 = np.asarray(b1, dtype=np.float32)
    wT0, wT1, biaspack = _host_prep(w0, b0, w1, b1)
    nc = _get()
    in_maps = [
        {
            "x": np.ascontiguousarray(x[b]),
            "wT0": wT0,
            "wT1": wT1,
            "biaspack": biaspack,
        }
        for b in range(B)
    ]
    res = run_bass_kernel_spmd(nc, in_maps, core_ids=list(range(B)))
    out = np.stack([res.results[b]["out"] for b in range(B)])          # (8, S, H)
    h1_last = np.stack([res.results[b]["hN0"].reshape(H) for b in range(B)])  # (8, H)
    h2_last = out[:, -1, :]                                            # (8, H)
    hN = np.stack([h1_last, h2_last])[:, :, None, :]                   # (2, 8, 1, H)
    return out, hN


# revision 62
# speedup vs baseline: 1.0231x; 1.0231x over previous
"""MinGRU (2-layer) Trainium2 Bass kernel.

Full shapes: x (8, 4096, 512) f32; w0,w1 (1024, 512); b0,b1 (1024,).
Per layer (reference semantics):
    gh = x @ w.T + b ; gate, hidden = split(gh, 2)
    z = sigmoid(gate); htil = g(hidden) = max(hidden+0.5, sigmoid(hidden))
    h_t = (1-z_t) h_{t-1} + z_t htil_t   (h_0 = 0.5)
Returns (h2, stack([h1[:, -1:], h2[:, -1:]])).

Sharding: data-parallel over batch B=8 across the 8 cores (one batch element
per core, no communication).  On-chip everything is laid out (channel, seq):
matmuls run in float32r (TF32-class accuracy at full PE rate), the gating
sigmoids on the scalar engine read straight out of PSUM with the bias folded
into the activation, and the recurrence is a single hardware
TensorTensorScan per (channel-group, chunk): state = a*state - negb with
negb = (a-1)*g.  x is transposed on-chip via PE matmul-transposes; weights
are pre-transposed (and biases pre-packed) on the host since they are tiny.
DMA instruction count is kept low (one chunk-sized DMA each way per chunk)
because each dma_start pays a fixed HWDGE issue cost.
"""

import os
import numpy as np

import concourse.bacc as bacc
import concourse.tile as tile
import concourse.mybir as mybir
from concourse.bass_utils import run_bass_kernel_spmd
from concourse.masks import make_identity

F32 = mybir.dt.float32
F32R = mybir.dt.float32r
AF = mybir.ActivationFunctionType
OP = mybir.AluOpType

P = 128          # partitions
S = 4096         # sequence length
D = 512          # input dim (layer-0 contraction)
H = 512          # hidden dim
CH = 512         # seq chunk (matmul N, one PSUM bank)
NCH = S // CH    # 8 chunks
KG = D // P      # 4 contraction groups
MT = (2 * H) // P  # 8 output m-tiles (4 gate + 4 hidden)
HG = H // P      # 4 hidden-channel groups
B = 8            # batch (= cores)

# hidden groups whose negb runs on GpSimd (rest on DVE) — load balance
NEGB_POOL_GROUPS = {0}


def build(reps: int = 1):
    nc = bacc.Bacc("TRN2", target_bir_lowering=False, debug=False)
    x_d = nc.dram_tensor("x", [S, D], F32, kind="ExternalInput")
    # host-pretransposed weights: wT[l][g, p, o] = w[l][o, g*128+p]
    wT_d = [
        nc.dram_tensor("wT0", [MT, P, KG, P], F32, kind="ExternalInput"),
        nc.dram_tensor("wT1", [MT, P, KG, P], F32, kind="ExternalInput"),
    ]
    # host-packed biases (128, 24): col = l*12 + kind*4 + i
    # kind: 0 = -b_gate, 1 = b_hidden, 2 = b_hidden+0.5
    bias_d = nc.dram_tensor("biaspack", [P, 2 * 3 * HG], F32, kind="ExternalInput")
    out_d = nc.dram_tensor("out", [S, H], F32, kind="ExternalOutput")
    hN0_d = nc.dram_tensor("hN0", [HG, P], F32, kind="ExternalOutput")

    with tile.TileContext(nc) as tc:
        with (
            tc.tile_pool(name="consts", bufs=1) as consts,
            tc.tile_pool(name="wp", bufs=2 * MT) as wp,
            tc.tile_pool(name="stage", bufs=2) as stage,
            tc.tile_pool(name="xt", bufs=8) as xtp,
            tc.tile_pool(name="h1", bufs=KG * NCH) as h1p,
            tc.tile_pool(name="gat", bufs=6) as gat,
            tc.tile_pool(name="tmp", bufs=3) as tmp,
            tc.tile_pool(name="h2", bufs=8) as h2p,
            tc.tile_pool(name="outT", bufs=2) as outTp,
            tc.tile_pool(name="ps_mm", bufs=8, space="PSUM") as ps_mm,
        ):
            ident = consts.tile([P, P], F32, tag="ident")
            make_identity(nc, ident[:])
            ident_r = consts.tile([P, P], F32R, tag="ident_r")
            nc.scalar.copy(ident_r[:], ident[:])

            btile = consts.tile([P, 2 * 3 * HG], F32, tag="btile")
            nc.sync.dma_start(btile[:], bias_d.ap())

            def bias_col(l, kind, i):
                c = (l * 3 + kind) * HG + i
                return btile[:, c : c + 1]

            wT = [[None] * MT for _ in range(2)]

            def load_w(l):
                # one DMA per m-tile: (128, KG, 128), all k-groups together
                for m in range(MT):
                    t = wp.tile([P, KG, P], F32R, tag="wT", name=f"wT{l}_{m}")
                    nc.sync.dma_start(t[:], wT_d[l].ap()[m].bitcast(F32R))
                    wT[l][m] = t

            def stage_x(c):
                # one DMA: (128, 4, 512) <- x rows [c*512, (c+1)*512)
                t = stage.tile([P, 4, D], F32R, tag="stage", name=f"xst{c}")
                nc.sync.dma_start(
                    t[:],
                    x_d.ap()[c * CH : (c + 1) * CH, :]
                    .rearrange("(j p) d -> p j d", p=P)
                    .bitcast(F32R),
                )
                return t

            def tr_copy(xst, split=False):
                # split=True: xst is a list of 4 per-subblock tiles (chunk 0
                # head-start: each transpose waits on 256KB, not 1MB)
                rhs = []
                for g in range(KG):
                    ptr = ps_mm.tile([P, CH], F32R, tag="gh")
                    for j in range(4):
                        src = (xst[j][:, g * P : (g + 1) * P] if split
                               else xst[:, j, g * P : (g + 1) * P])
                        nc.tensor.transpose(
                            ptr[:, j * P : (j + 1) * P], src, ident_r[:],
                        )
                    xt = xtp.tile([P, CH], F32R, tag="xt")
                    nc.scalar.copy(xt[:], ptr[:])
                    rhs.append(xt)
                return rhs

            def emit_out(c, h2_tiles, last=False):
                ot = outTp.tile([P, 4, H], F32, tag="outT", name=f"ot{c}")
                for j in range(4):
                    ptr = ps_mm.tile([P, CH], F32R, tag="gh", name=f"optr{c}_{j}")
                    for i in range(HG):
                        nc.tensor.transpose(
                            ptr[:, i * P : (i + 1) * P],
                            h2_tiles[i][c][:, j * P : (j + 1) * P],
                            ident_r[:],
                        )
                    if last and j % 2 == 1:
                        nc.vector.tensor_copy(ot[:, j, :], ptr[:])
                    else:
                        nc.scalar.copy(ot[:, j, :], ptr[:])
                    nc.sync.dma_start(
                        out_d.ap()[(c * 4 + j) * P : (c * 4 + j + 1) * P, :],
                        ot[:, j, :],
                    )

            first = True
            LAG = 2  # layer-1 trails layer-0 by this many chunks
            for _rep in range(reps):
                h1_tiles = [[None] * NCH for _ in range(HG)]
                h2_tiles = [[None] * NCH for _ in range(HG)]
                out_pending = []
                xst_q = {}
                rhs_q = {}
                steps = []
                for k in range(NCH + LAG):
                    if k < NCH:
                        steps.append((0, k))
                    if k - LAG >= 0:
                        steps.append((1, k - LAG))
                for l, c in steps:
                        if first:
                            xst0 = []
                            for j in range(4):
                                t0 = stage.tile([P, D], F32R, tag="st0", name=f"xst0_{j}")
                                nc.sync.dma_start(
                                    t0[:],
                                    x_d.ap()[j * P : (j + 1) * P, :].bitcast(F32R),
                                )
                                xst0.append(t0)
                            load_w(0)
                            xst_q[1] = stage_x(1)
                            rhs_q[0] = tr_copy(xst0, split=True)
                            first = False
                        if l == 0 and c == 1:
                            load_w(1)

                        # ---- rhs tiles (128 k, CH) float32r
                        if l == 0:
                            # prefetch DMA two chunks ahead
                            if c + 2 < NCH and (c + 2) not in xst_q:
                                xst_q[c + 2] = stage_x(c + 2)
                            rhs = rhs_q.pop(c)
                        else:
                            rhs = [h1_tiles[g][c] for g in range(KG)]

                        # ---- matmuls: gh m-tiles (128 o, CH s) in PSUM
                        gh = []
                        for m in range(MT):
                            pt = ps_mm.tile([P, CH], F32, tag="gh")
                            for g in range(KG):
                                nc.tensor.matmul(
                                    pt[:],
                                    wT[l][m][:, g, :],
                                    rhs[g][:],
                                    start=(g == 0),
                                    stop=(g == KG - 1),
                                )
                            gh.append(pt)

                        # transpose the next x chunk right after this step's
                        # matmuls (PE spacer while ACT drains the gh tiles)
                        if l == 0 and c + 1 < NCH and (c + 1) not in rhs_q:
                            rhs_q[c + 1] = tr_copy(xst_q.pop(c + 1))

                        # ---- gating + scan per hidden group
                        for i in range(HG):
                            a = gat.tile([P, CH], F32, tag="a")
                            nc.scalar.activation(
                                a[:], gh[i][:], AF.Sigmoid,
                                bias=bias_col(l, 0, i), scale=-1.0,
                            )
                            sh = tmp.tile([P, CH], F32, tag="sh")
                            nc.scalar.activation(
                                sh[:], gh[HG + i][:], AF.Sigmoid,
                                bias=bias_col(l, 1, i), scale=1.0,
                            )
                            gt = tmp.tile([P, CH], F32, tag="g")
                            nc.vector.scalar_tensor_tensor(
                                gt[:], gh[HG + i][:], bias_col(l, 2, i), sh[:],
                                OP.add, OP.max,
                            )
                            negb = gat.tile([P, CH], F32, tag="negb")
                            if i in NEGB_POOL_GROUPS and not (l == 1 and c == NCH - 1):
                                # Pool supports only plain tensor_tensor:
                                # negb = a*g - g = (a-1)*g
                                ag = tmp.tile([P, CH], F32, tag="ag")
                                nc.gpsimd.tensor_tensor(ag[:], a[:], gt[:], OP.mult)
                                nc.gpsimd.tensor_tensor(negb[:], ag[:], gt[:], OP.subtract)
                            else:
                                nc.vector.scalar_tensor_tensor(
                                    negb[:], a[:], 1.0, gt[:], OP.subtract, OP.mult
                                )
                            if l == 0:
                                h = h1p.tile([P, CH], F32R, tag="h1")
                                h1_tiles[i][c] = h
                                init = 0.5 if c == 0 else h1_tiles[i][c - 1][:, CH - 1 : CH].bitcast(F32)
                            else:
                                h = h2p.tile([P, CH], F32R, tag="h2")
                                h2_tiles[i][c] = h
                                init = 0.5 if c == 0 else h2_tiles[i][c - 1][:, CH - 1 : CH].bitcast(F32)
                            nc.vector.tensor_tensor_scan(
                                h[:], a[:], negb[:], init, OP.mult, OP.subtract
                            )

                        # ---- layer-1 output: transpose back to (s, h), store
                        # chunk c-1 (delayed so PE never waits on the scans)
                        if l == 1:
                            out_pending.append(c)
                            if len(out_pending) > 2:
                                emit_out(out_pending.pop(0), h2_tiles)

                while out_pending:
                    c_out = out_pending.pop(0)
                    emit_out(c_out, h2_tiles, last=not out_pending)

                # ---- next-hidden for layer 0 (layer 1's comes from out[:, -1, :])
                hg = consts.tile([P, HG], F32, tag="hgather", name=f"hg{_rep}")
                for i in range(HG):
                    nc.scalar.copy(
                        hg[:, i : i + 1],
                        h1_tiles[i][NCH - 1][:, CH - 1 : CH].bitcast(F32),
                    )
                pgt = ps_mm.tile([HG, P], F32, tag="gh", name=f"pg{_rep}")
                nc.tensor.transpose(pgt[:], hg[:], ident[:])
                hgt = consts.tile([HG, P], F32, tag="hgathT", name=f"hgt{_rep}")
                nc.scalar.copy(hgt[:], pgt[:])
                nc.sync.dma_start(hN0_d.ap(), hgt[:])
    nc.compile()
    return nc


_cache = {}


def _get(reps=1):
    if reps not in _cache:
        _cache[reps] = build(reps)
    return _cache[reps]


def _host_prep(w0, b0, w1, b1):
    """Pre-transpose weights into (KG, 128, 2H) and pack biases (128, 24)."""
    wts = []
    for w in (w0, w1):
        # w.T is (D, 2H): [g*128+p, m*128+o'] -> (MT, P, KG, P) m-major tiles
        wt = np.ascontiguousarray(
            w.T.reshape(KG, P, MT, P).transpose(2, 1, 0, 3)
        )
        wts.append(wt.astype(np.float32))
    bias = np.empty((2, 3, P, HG), np.float32)
    for l, b in enumerate((b0, b1)):
        bg = b[:H].reshape(HG, P).T
        bh = b[H:].reshape(HG, P).T
        bias[l, 0] = -bg
        bias[l, 1] = bh
        bias[l, 2] = bh + 0.5
    biaspack = np.ascontiguousarray(bias.transpose(2, 0, 1, 3).reshape(P, 2 * 3 * HG))
    return wts[0], wts[1], biaspack


def kernel(x, w0, b0, w1, b1):
    x = np.asarray(x, dtype=np.float32)
    w0 = np.asarray(w0, dtype=np.float32)
    b0 = np.asarray(b0, dtype=np.float32)
    w1 = np.asarray(w1, dtype=np.float32)
    b1 = np.asarray(b1, dtype=np.float32)
    wT0, wT1, biaspack = _host_prep(w0, b0, w1, b1)
    nc = _get()
    in_maps = [
        {
            "x": np.ascontiguousarray(x[b]),
            "wT0": wT0,
            "wT1": wT1,
            "biaspack": biaspack,
        }
        for b in range(B)
    ]
    res = run_bass_kernel_spmd(nc, in_maps, core_ids=list(range(B)))
    out = np.stack([res.results[b]["out"] for b in range(B)])          # (8, S, H)
    h1_last = np.stack([res.results[b]["hN0"].reshape(H) for b in range(B)])  # (8, H)
    h2_last = out[:, -1, :]                                            # (8, H)
    hN = np.stack([h1_last, h2_last])[:, :, None, :]                   # (2, 8, 1, H)
    return out, hN


# revision 63
# speedup vs baseline: 1.0393x; 1.0158x over previous
"""MinGRU (2-layer) Trainium2 Bass kernel.

Full shapes: x (8, 4096, 512) f32; w0,w1 (1024, 512); b0,b1 (1024,).
Per layer (reference semantics):
    gh = x @ w.T + b ; gate, hidden = split(gh, 2)
    z = sigmoid(gate); htil = g(hidden) = max(hidden+0.5, sigmoid(hidden))
    h_t = (1-z_t) h_{t-1} + z_t htil_t   (h_0 = 0.5)
Returns (h2, stack([h1[:, -1:], h2[:, -1:]])).

Sharding: data-parallel over batch B=8 across the 8 cores (one batch element
per core, no communication).  On-chip everything is laid out (channel, seq):
matmuls run in float32r (TF32-class accuracy at full PE rate), the gating
sigmoids on the scalar engine read straight out of PSUM with the bias folded
into the activation, and the recurrence is a single hardware
TensorTensorScan per (channel-group, chunk): state = a*state - negb with
negb = (a-1)*g.  x is transposed on-chip via PE matmul-transposes; weights
are pre-transposed (and biases pre-packed) on the host since they are tiny.
DMA instruction count is kept low (one chunk-sized DMA each way per chunk)
because each dma_start pays a fixed HWDGE issue cost.
"""

import os
import numpy as np

import concourse.bacc as bacc
import concourse.tile as tile
import concourse.mybir as mybir
from concourse.bass_utils import run_bass_kernel_spmd
from concourse.masks import make_identity

F32 = mybir.dt.float32
F32R = mybir.dt.float32r
AF = mybir.ActivationFunctionType
OP = mybir.AluOpType

P = 128          # partitions
S = 4096         # sequence length
D = 512          # input dim (layer-0 contraction)
H = 512          # hidden dim
CH = 512         # seq chunk (matmul N, one PSUM bank)
NCH = S // CH    # 8 chunks
KG = D // P      # 4 contraction groups
MT = (2 * H) // P  # 8 output m-tiles (4 gate + 4 hidden)
HG = H // P      # 4 hidden-channel groups
B = 8            # batch (= cores)

# hidden groups whose negb runs on GpSimd (rest on DVE) — load balance
NEGB_POOL_GROUPS = set()


def build(reps: int = 1):
    nc = bacc.Bacc("TRN2", target_bir_lowering=False, debug=False)
    x_d = nc.dram_tensor("x", [S, D], F32, kind="ExternalInput")
    # host-pretransposed weights: wT[l][g, p, o] = w[l][o, g*128+p]
    wT_d = [
        nc.dram_tensor("wT0", [MT, P, KG, P], F32, kind="ExternalInput"),
        nc.dram_tensor("wT1", [MT, P, KG, P], F32, kind="ExternalInput"),
    ]
    # host-packed biases (128, 24): col = l*12 + kind*4 + i
    # kind: 0 = -b_gate, 1 = b_hidden, 2 = b_hidden+0.5
    bias_d = nc.dram_tensor("biaspack", [P, 2 * 3 * HG], F32, kind="ExternalInput")
    out_d = nc.dram_tensor("out", [S, H], F32, kind="ExternalOutput")
    hN0_d = nc.dram_tensor("hN0", [HG, P], F32, kind="ExternalOutput")

    with tile.TileContext(nc) as tc:
        with (
            tc.tile_pool(name="consts", bufs=1) as consts,
            tc.tile_pool(name="wp", bufs=2 * MT) as wp,
            tc.tile_pool(name="stage", bufs=2) as stage,
            tc.tile_pool(name="xt", bufs=8) as xtp,
            tc.tile_pool(name="h1", bufs=KG * NCH) as h1p,
            tc.tile_pool(name="gat", bufs=6) as gat,
            tc.tile_pool(name="tmp", bufs=3) as tmp,
            tc.tile_pool(name="h2", bufs=8) as h2p,
            tc.tile_pool(name="outT", bufs=2) as outTp,
            tc.tile_pool(name="ps_mm", bufs=8, space="PSUM") as ps_mm,
        ):
            ident = consts.tile([P, P], F32, tag="ident")
            make_identity(nc, ident[:])
            ident_r = consts.tile([P, P], F32R, tag="ident_r")
            nc.scalar.copy(ident_r[:], ident[:])

            btile = consts.tile([P, 2 * 3 * HG], F32, tag="btile")
            nc.sync.dma_start(btile[:], bias_d.ap())

            def bias_col(l, kind, i):
                c = (l * 3 + kind) * HG + i
                return btile[:, c : c + 1]

            wT = [[None] * MT for _ in range(2)]

            def load_w(l):
                # one DMA per m-tile: (128, KG, 128), all k-groups together
                for m in range(MT):
                    t = wp.tile([P, KG, P], F32R, tag="wT", name=f"wT{l}_{m}")
                    nc.sync.dma_start(t[:], wT_d[l].ap()[m].bitcast(F32R))
                    wT[l][m] = t

            def stage_x(c):
                # one DMA: (128, 4, 512) <- x rows [c*512, (c+1)*512)
                t = stage.tile([P, 4, D], F32R, tag="stage", name=f"xst{c}")
                nc.sync.dma_start(
                    t[:],
                    x_d.ap()[c * CH : (c + 1) * CH, :]
                    .rearrange("(j p) d -> p j d", p=P)
                    .bitcast(F32R),
                )
                return t

            def tr_copy(xst, split=False):
                # split=True: xst is a list of 4 per-subblock tiles (chunk 0
                # head-start: each transpose waits on 256KB, not 1MB)
                rhs = []
                for g in range(KG):
                    ptr = ps_mm.tile([P, CH], F32R, tag="gh")
                    for j in range(4):
                        src = (xst[j][:, g * P : (g + 1) * P] if split
                               else xst[:, j, g * P : (g + 1) * P])
                        nc.tensor.transpose(
                            ptr[:, j * P : (j + 1) * P], src, ident_r[:],
                        )
                    xt = xtp.tile([P, CH], F32R, tag="xt")
                    nc.scalar.copy(xt[:], ptr[:])
                    rhs.append(xt)
                return rhs

            def emit_out(c, h2_tiles, last=False):
                ot = outTp.tile([P, 4, H], F32, tag="outT", name=f"ot{c}")
                for j in range(4):
                    ptr = ps_mm.tile([P, CH], F32R, tag="gh", name=f"optr{c}_{j}")
                    for i in range(HG):
                        nc.tensor.transpose(
                            ptr[:, i * P : (i + 1) * P],
                            h2_tiles[i][c][:, j * P : (j + 1) * P],
                            ident_r[:],
                        )
                    if last and j % 2 == 1:
                        nc.vector.tensor_copy(ot[:, j, :], ptr[:])
                    else:
                        nc.scalar.copy(ot[:, j, :], ptr[:])
                    nc.sync.dma_start(
                        out_d.ap()[(c * 4 + j) * P : (c * 4 + j + 1) * P, :],
                        ot[:, j, :],
                    )

            first = True
            LAG = 2  # layer-1 trails layer-0 by this many chunks
            for _rep in range(reps):
                h1_tiles = [[None] * NCH for _ in range(HG)]
                h2_tiles = [[None] * NCH for _ in range(HG)]
                out_pending = []
                xst_q = {}
                rhs_q = {}
                steps = []
                for k in range(NCH + LAG):
                    if k < NCH:
                        steps.append((0, k))
                    if k - LAG >= 0:
                        steps.append((1, k - LAG))
                for l, c in steps:
                        if first:
                            xst0 = []
                            for j in range(4):
                                t0 = stage.tile([P, D], F32R, tag="st0", name=f"xst0_{j}")
                                nc.sync.dma_start(
                                    t0[:],
                                    x_d.ap()[j * P : (j + 1) * P, :].bitcast(F32R),
                                )
                                xst0.append(t0)
                            load_w(0)
                            xst_q[1] = stage_x(1)
                            rhs_q[0] = tr_copy(xst0, split=True)
                            first = False
                        if l == 0 and c == 1:
                            load_w(1)

                        # ---- rhs tiles (128 k, CH) float32r
                        if l == 0:
                            # prefetch DMA two chunks ahead
                            if c + 2 < NCH and (c + 2) not in xst_q:
                                xst_q[c + 2] = stage_x(c + 2)
                            rhs = rhs_q.pop(c)
                        else:
                            rhs = [h1_tiles[g][c] for g in range(KG)]

                        # ---- matmuls: gh m-tiles (128 o, CH s) in PSUM
                        gh = []
                        for m in range(MT):
                            pt = ps_mm.tile([P, CH], F32, tag="gh")
                            for g in range(KG):
                                nc.tensor.matmul(
                                    pt[:],
                                    wT[l][m][:, g, :],
                                    rhs[g][:],
                                    start=(g == 0),
                                    stop=(g == KG - 1),
                                )
                            gh.append(pt)

                        # transpose the next x chunk right after this step's
                        # matmuls (PE spacer while ACT drains the gh tiles)
                        if l == 0 and c + 1 < NCH and (c + 1) not in rhs_q:
                            rhs_q[c + 1] = tr_copy(xst_q.pop(c + 1))

                        # ---- gating + scan per hidden group
                        for i in range(HG):
                            a = gat.tile([P, CH], F32, tag="a")
                            nc.scalar.activation(
                                a[:], gh[i][:], AF.Sigmoid,
                                bias=bias_col(l, 0, i), scale=-1.0,
                            )
                            sh = tmp.tile([P, CH], F32, tag="sh")
                            nc.scalar.activation(
                                sh[:], gh[HG + i][:], AF.Sigmoid,
                                bias=bias_col(l, 1, i), scale=1.0,
                            )
                            gt = tmp.tile([P, CH], F32, tag="g")
                            nc.vector.scalar_tensor_tensor(
                                gt[:], gh[HG + i][:], bias_col(l, 2, i), sh[:],
                                OP.add, OP.max,
                            )
                            negb = gat.tile([P, CH], F32, tag="negb")
                            if i in NEGB_POOL_GROUPS and not (l == 1 and c == NCH - 1):
                                # Pool supports only plain tensor_tensor:
                                # negb = a*g - g = (a-1)*g
                                ag = tmp.tile([P, CH], F32, tag="ag")
                                nc.gpsimd.tensor_tensor(ag[:], a[:], gt[:], OP.mult)
                                nc.gpsimd.tensor_tensor(negb[:], ag[:], gt[:], OP.subtract)
                            else:
                                nc.vector.scalar_tensor_tensor(
                                    negb[:], a[:], 1.0, gt[:], OP.subtract, OP.mult
                                )
                            if l == 0:
                                h = h1p.tile([P, CH], F32R, tag="h1")
                                h1_tiles[i][c] = h
                                init = 0.5 if c == 0 else h1_tiles[i][c - 1][:, CH - 1 : CH].bitcast(F32)
                            else:
                                h = h2p.tile([P, CH], F32R, tag="h2")
                                h2_tiles[i][c] = h
                                init = 0.5 if c == 0 else h2_tiles[i][c - 1][:, CH - 1 : CH].bitcast(F32)
                            nc.vector.tensor_tensor_scan(
                                h[:], a[:], negb[:], init, OP.mult, OP.subtract
                            )

                        # ---- layer-1 output: transpose back to (s, h), store
                        # chunk c-1 (delayed so PE never waits on the scans)
                        if l == 1:
                            out_pending.append(c)
                            if len(out_pending) > 2:
                                emit_out(out_pending.pop(0), h2_tiles)

                while out_pending:
                    c_out = out_pending.pop(0)
                    emit_out(c_out, h2_tiles, last=not out_pending)

                # ---- next-hidden for layer 0 (layer 1's comes from out[:, -1, :])
                hg = consts.tile([P, HG], F32, tag="hgather", name=f"hg{_rep}")
                for i in range(HG):
                    nc.scalar.copy(
                        hg[:, i : i + 1],
                        h1_tiles[i][NCH - 1][:, CH - 1 : CH].bitcast(F32),
                    )
                pgt = ps_mm.tile([HG, P], F32, tag="gh", name=f"pg{_rep}")
                nc.tensor.transpose(pgt[:], hg[:], ident[:])
                hgt = consts.tile([HG, P], F32, tag="hgathT", name=f"hgt{_rep}")
                nc.scalar.copy(hgt[:], pgt[:])
                nc.sync.dma_start(hN0_d.ap(), hgt[:])
    nc.compile()
    return nc


_cache = {}


def _get(reps=1):
    if reps not in _cache:
        _cache[reps] = build(reps)
    return _cache[reps]


def _host_prep(w0, b0, w1, b1):
    """Pre-transpose weights into (KG, 128, 2H) and pack biases (128, 24)."""
    wts = []
    for w in (w0, w1):
        # w.T is (D, 2H): [g*128+p, m*128+o'] -> (MT, P, KG, P) m-major tiles
        wt = np.ascontiguousarray(
            w.T.reshape(KG, P, MT, P).transpose(2, 1, 0, 3)
        )
        wts.append(wt.astype(np.float32))
    bias = np.empty((2, 3, P, HG), np.float32)
    for l, b in enumerate((b0, b1)):
        bg = b[:H].reshape(HG, P).T
        bh = b[H:].reshape(HG, P).T
        bias[l, 0] = -bg
        bias[l, 1] = bh
        bias[l, 2] = bh + 0.5
    biaspack = np.ascontiguousarray(bias.transpose(2, 0, 1, 3).reshape(P, 2 * 3 * HG))
    return wts[0], wts[1], biaspack


def kernel(x, w0, b0, w1, b1):
    x = np.asarray(x, dtype=np.float32)
    w0 = np.asarray(w0, dtype=np.float32)
    b0 = np.asarray(b0, dtype=np.float32)
    w1 = np.asarray(w1, dtype=np.float32)
    b1 = np.asarray(b1, dtype=np.float32)
    wT0, wT1, biaspack = _host_prep(w0, b0, w1, b1)
    nc = _get()
    in_maps = [
        {
            "x": np.ascontiguousarray(x[b]),
            "wT0": wT0,
            "wT1": wT1,
            "biaspack": biaspack,
        }
        for b in range(B)
    ]
    res = run_bass_kernel_spmd(nc, in_maps, core_ids=list(range(B)))
    out = np.stack([res.results[b]["out"] for b in range(B)])          # (8, S, H)
    h1_last = np.stack([res.results[b]["hN0"].reshape(H) for b in range(B)])  # (8, H)
    h2_last = out[:, -1, :]                                            # (8, H)
    hN = np.stack([h1_last, h2_last])[:, :, None, :]                   # (2, 8, 1, H)
    return out, hN
